# revision 31
# baseline (speedup 1.0000x reference)
"""Causal self-attention (B=2, T=4096, C=768, H=12, D=64, RoPE) on 8 TRN2 cores.

Sharding: core c handles batch b = c//4 and heads [3g, 3g+1, 3g+2] with g = c%4
(data parallel over B, tensor parallel over heads). Each core computes its
heads' QKV projections, RoPE, causal attention and the partial output
projection; the host sums the 4 partial projections per batch.

Device-side layouts (per core):
  - x is shipped transposed: xT [768, 4096]; cast twice on device: fp8e4
    (q/k projections) and bf16 (v projection).
  - q/k projections run fp8e4 DoubleRow (0.5 cycles/row, 3 plane pairs over
    the 768 contraction); weights host-scaled by 64 for fp8 range, the 1/64
    folded into the RoPE tables. v projection runs bf16 (fp8 weight error
    would pass straight through attention into the output).
  - RoPE uses per-head permuted channel order [even d | odd d]: full-width
    elementwise ops + a 32-partition swap via SBUF->SBUF DMA; cos-mult and
    add on DVE, sin-mult on GPSIMD; q/k stored bf16.
  - attention computes S^T (keys on partitions, queries free) in bf16 with
    causal column trimming on diagonal key chunks; exp on the scalar engine
    from PSUM to bf16 P values; the remaining [128,128] diagonal triangle is
    masked by one shared mask multiply. P^T @ V is bf16 with a ones-augmented
    V (65 cols/head): PSUM row 64 accumulates the softmax denominator.
    1/denominator via reciprocal_approx_fast + gpsimd partition broadcast.
  - the output projection is f32r over 2 planes of 128 v-channels (plane 1
    zero-padded); y stored f32r.
"""

import sys

sys.path.insert(0, "/opt/trn_rl_repo")

from contextlib import ExitStack

import numpy as np

import concourse.bass as bass
import concourse.tile as tile
from concourse import bacc, mybir
from concourse.bass_utils import run_bass_kernel_spmd
from concourse.masks import make_identity

P = 128
C = 768
D = 64
HPC = 3            # heads per core
DQ = HPC * D       # 192 channels per core
WQK = 2 * DQ       # 384 q+k output channels per core
KCH = 6            # contraction chunks of 128 (3 DoubleRow pairs)
TCH = 1024         # phase B column chunk
SCQ = 512          # query superchunk (attention free dim)
GK = 2             # S^T tiles per exp group ([128, 1024] PSUM = 2 banks)
VW = HPC * 65      # v_aug row width per key chunk (3 heads x (64 + ones))
PCH = 512          # projection column chunk

f32 = mybir.dt.float32
f32r = mybir.dt.float32r
bf16 = mybir.dt.bfloat16
f8 = mybir.dt.float8e4
EXP = mybir.ActivationFunctionType.Exp
DR = mybir.MatmulPerfMode.DoubleRow

WS = 64.0     # q/k weight prescale (fp8 range); folded into rope tables


def build(T=4096, n_cores=8):
    NT = T // TCH          # phase B chunks
    NSC = T // SCQ         # query superchunks
    nc = bacc.Bacc("TRN2", target_bir_lowering=False, debug=False,
                   num_devices=n_cores)

    xT_d = nc.dram_tensor("xT", [C, T], bf16, kind="ExternalInput").ap()
    wqk_d = nc.dram_tensor("wqk", [C, WQK], bf16, kind="ExternalInput").ap()
    wv_d = nc.dram_tensor("wv", [C, DQ], bf16, kind="ExternalInput").ap()
    wp_d = nc.dram_tensor("wp", [P, 2 * C], f32, kind="ExternalInput").ap()
    cp_d = nc.dram_tensor("cpat", [P, T], bf16, kind="ExternalInput").ap()
    sp_d = nc.dram_tensor("spat", [P, T], bf16, kind="ExternalInput").ap()
    mk_d = nc.dram_tensor("mk", [P, P], bf16, kind="ExternalInput").ap()
    out_d = nc.dram_tensor("outT", [C, T], f32, kind="ExternalOutput").ap()

    xT_v = xT_d.rearrange("(a p) t -> p a t", p=P)

    with tile.TileContext(nc) as tc, ExitStack() as top:
        const = top.enter_context(tc.tile_pool(name="const", bufs=1))
        persist = top.enter_context(tc.tile_pool(name="persist", bufs=1))
        ptp = top.enter_context(tc.tile_pool(name="ptp", bufs=4))
        smp = top.enter_context(tc.tile_pool(name="smp", bufs=2))
        otp = top.enter_context(tc.tile_pool(name="otp", bufs=3))

        # --- constants ---
        wqk_f8 = const.tile([P, KCH, WQK], f8)
        wv_bf = const.tile([P, KCH, DQ], bf16)
        wp_r = const.tile([P, 2, C], f32r)
        cpb = const.tile([P, T], bf16)
        spb = const.tile([P, T], bf16)
        mk128 = const.tile([P, P], bf16)

        # --- persistent activations ---
        q01 = persist.tile([P, T], bf16)      # q heads 0,1
        k01 = persist.tile([P, T], bf16)      # k heads 0,1
        q2 = persist.tile([D, T], bf16)       # q head 2
        k2 = persist.tile([D, T], bf16)       # k head 2
        v_aug = persist.tile([P, (T // P) * VW], bf16)
        yt = persist.tile([P, 2, T], f32r)    # y, proj plane layout

        # zero the unused proj plane rows (avoid NaN garbage)
        nc.gpsimd.memset(yt[D:P, 1, :].bitcast(f32), 0.0)

        # ones columns of v_aug
        ones_view = v_aug[:].rearrange(
            "p (a h c) -> p a h c", h=HPC, c=65)[:, :, :, 64]
        ones_f = const.tile([P, (T // P) * HPC], f32)
        nc.gpsimd.memset(ones_f[:], 1.0)
        nc.vector.tensor_copy(ones_view,
                              ones_f[:].rearrange("p (a h) -> p a h", h=HPC))

        # --- load + cast weights / tables ---
        def emit_consts(ldp):
          for kc in range(KCH):
            wtmp = ldp.tile([P, WQK], bf16, tag="wtmp")
            nc.sync.dma_start(wtmp[:], wqk_d[kc * P:(kc + 1) * P, :])
            nc.vector.tensor_copy(wqk_f8[:, kc, :], wtmp[:])
            nc.sync.dma_start(wv_bf[:, kc, :], wv_d[kc * P:(kc + 1) * P, :])
          nc.sync.dma_start(cpb[:], cp_d[:])
          nc.sync.dma_start(spb[:], sp_d[:])
          nc.sync.dma_start(mk128[:], mk_d[:])
          wptmp = ldp.tile([P, 2 * C], f32, tag="wptmp")
          nc.sync.dma_start(wptmp[:], wp_d[:, :])
          nc.vector.tensor_copy(
              wp_r[:].rearrange("p a c -> p (a c)"), wptmp[:])

        # m-chunks: (offset, rows, dst)
        qk_mchunks = [
            (0, P, q01), (P, D, q2), (DQ, P, k01), (DQ + P, D, k2),
        ]
        v_kc = v_aug[:].rearrange("p (a w) -> p a w", w=VW)
        va4 = v_aug[:].rearrange("p (a h c) -> p a h c", h=HPC, c=65)

        def emit_b(n, xp, rtmp, qkv_ps):
            cols = slice(n * TCH, (n + 1) * TCH)
            xb = xp.tile([P, KCH, TCH], bf16, tag="xb")
            nc.sync.dma_start(xb[:], xT_v[:, :, cols])
            xr8 = xp.tile([P, KCH, TCH], f8, tag="xr8")
            nc.scalar.copy(xr8[:].rearrange("p a t -> p (a t)"),
                           xb[:].rearrange("p a t -> p (a t)"))

            for moff, rows, dst in qk_mchunks:
                ps = qkv_ps.tile([rows, TCH], f32, tag="qk")
                for half in range(TCH // 256):
                    hs = slice(half * 256, half * 256 + 256)
                    for j in range(KCH // 2):
                        nc.tensor.matmul(
                            ps[:, hs],
                            wqk_f8[:, 2 * j:2 * j + 2, moff:moff + rows],
                            xr8[:, 2 * j:2 * j + 2, hs],
                            start=(j == 0), stop=(j == KCH // 2 - 1),
                            perf_mode=DR)
                # rope: out = psum*cpat + swap32(psum*spat)
                ct = rtmp.tile([rows, TCH], bf16, tag=f"ct{rows}")
                st = rtmp.tile([rows, TCH], bf16, tag=f"st{rows}")
                wt = rtmp.tile([rows, TCH], bf16, tag=f"wt{rows}")
                nc.vector.tensor_mul(ct[:], ps[:], cpb[0:rows, cols])
                nc.vector.tensor_mul(st[:], ps[:], spb[0:rows, cols])
                for blk in range(rows // D):
                    p0 = blk * D
                    nc.sync.dma_start(wt[p0:p0 + 32, :],
                                      st[p0 + 32:p0 + D, :])
                    nc.sync.dma_start(wt[p0 + 32:p0 + D, :],
                                      st[p0:p0 + 32, :])
                nc.gpsimd.tensor_add(dst[:, cols], ct[:], wt[:])

            # v^T directly: out[t, c] = sum_ch x[ch, t] * WvT[ch, c]
            for tt in range(TCH // P):
                kc32 = n * (TCH // P) + tt
                vps = qkv_ps.tile([P, DQ], f32, tag="vt")
                for kc in range(KCH):
                    nc.tensor.matmul(
                        vps[:], xb[:, kc, tt * P:(tt + 1) * P],
                        wv_bf[:, kc, :],
                        start=(kc == 0), stop=(kc == KCH - 1))
                nc.scalar.copy(
                    va4[:, kc32, :, 0:64],
                    vps[:].rearrange("p (h c) -> p h c", h=HPC))

        def emit_c(s, s_ps, y_ps, o_ps):
            scols = slice(s * SCQ, (s + 1) * SCQ)
            for h in range(HPC):
                if h < 2:
                    q_rows = q01[h * D:(h + 1) * D, :]
                    k_rows = k01[h * D:(h + 1) * D, :]
                else:
                    q_rows = q2[:, :]
                    k_rows = k2[:, :]
                psy = y_ps.tile([65, SCQ], f32, tag="y")
                ng = 2 * s + 2          # GK=2 chunk groups
                pending = None          # (pt, g) awaiting PV
                for g in range(ng):
                    pss = s_ps.tile([P, GK * SCQ], f32, tag="ss")
                    pt = ptp.tile([P, GK * SCQ], bf16, tag="pt")
                    offs = []
                    for j in range(GK):
                        kj = 2 * g + j
                        off = P * (kj - 4 * s) if kj >= 4 * s else 0
                        offs.append(off)
                        nc.tensor.matmul(
                            pss[:, j * SCQ + off:(j + 1) * SCQ],
                            k_rows[:, kj * P:(kj + 1) * P],
                            q_rows[:, s * SCQ + off:(s + 1) * SCQ],
                            start=True, stop=True)
                    if offs[0] == 0 and offs[1] == 0:
                        nc.scalar.activation(pt[:], pss[:], EXP,
                                             scale=0.125)
                    else:
                        for j in range(GK):
                            c0 = j * SCQ + offs[j]
                            c1 = (j + 1) * SCQ
                            nc.scalar.activation(pt[:, c0:c1],
                                                 pss[:, c0:c1], EXP,
                                                 scale=0.125)
                    for j in range(GK):
                        kj = 2 * g + j
                        if kj >= 4 * s:
                            c0 = j * SCQ + offs[j]
                            nc.vector.tensor_mul(
                                pt[:, c0:c0 + P], pt[:, c0:c0 + P],
                                mk128[:])
                    if pending is not None:
                        _emit_pv(nc, psy, v_kc, pending, h, ng, s)
                    pending = (pt, g)
                _emit_pv(nc, psy, v_kc, pending, h, ng, s)

                dr_t = smp.tile([1, SCQ], f32, tag="dr")
                nc.vector.tensor_copy(dr_t[:], psy[64:65, :])
                rf = smp.tile([1, SCQ], f32, tag="rf")
                nc.vector.reciprocal_approx_fast(rf[:], dr_t[:])
                rb = smp.tile([D, SCQ], f32, tag="rb")
                nc.gpsimd.partition_broadcast(rb[:], rf[:])
                if h < 2:
                    ydst = yt[h * D:(h + 1) * D, 0, scols]
                else:
                    ydst = yt[0:D, 1, scols]
                nc.vector.tensor_mul(ydst, psy[0:D, :], rb[:])

            # projection for this superchunk: out rows m*128, cols scols
            c0 = s * SCQ
            for m in range(C // P):
                pso = o_ps.tile([P, PCH], f32, tag="o")
                for i in range(2):
                    nc.tensor.matmul(
                        pso[:], wp_r[:, i, m * P:(m + 1) * P],
                        yt[:, i, c0:c0 + PCH],
                        start=(i == 0), stop=(i == 1))
                ot = otp.tile([P, PCH], f32, tag="ot")
                nc.vector.tensor_copy(ot[:], pso[:])
                nc.sync.dma_start(out_d[m * P:(m + 1) * P,
                                        c0:c0 + PCH], ot[:])

        with ExitStack() as bctx:
            ldp = bctx.enter_context(tc.tile_pool(name="ldp", bufs=2))
            xp = bctx.enter_context(tc.tile_pool(name="xp", bufs=2))
            rtmp = bctx.enter_context(tc.tile_pool(name="rtmp", bufs=2))
            qkv_ps = bctx.enter_context(
                tc.tile_pool(name="qkv_ps", bufs=2, space="PSUM"))
            emit_consts(ldp)
            for n in range(NT):
                emit_b(n, xp, rtmp, qkv_ps)
        with ExitStack() as cctx:
            s_ps = cctx.enter_context(
                tc.tile_pool(name="s_ps", bufs=2, space="PSUM"))
            y_ps = cctx.enter_context(
                tc.tile_pool(name="y_ps", bufs=2, space="PSUM"))
            o_ps = cctx.enter_context(
                tc.tile_pool(name="o_ps", bufs=2, space="PSUM"))
            for s in range(NSC):
                emit_c(s, s_ps, y_ps, o_ps)

    nc.compile()
    return nc


def _emit_pv(nc, psy, v_kc, pending, h, ng, s):
    pt, g = pending
    for j in range(GK):
        kj = 2 * g + j
        off = P * (kj - 4 * s) if kj >= 4 * s else 0
        nc.tensor.matmul(psy[:, off:SCQ],
                         v_kc[:, kj, h * 65:(h + 1) * 65],
                         pt[:, j * SCQ + off:(j + 1) * SCQ],
                         start=(kj == 0), stop=(kj == 2 * ng - 1))


# ---------------------------------------------------------------------------
# host side
# ---------------------------------------------------------------------------


def make_core_inputs(x, Wq, bq, Wk, bk, Wv, bv, Wp, bp, T=4096, n_cores=8):
    """Build the per-core input maps. Biases bq/bk/bv must be zero (they are
    for this problem); bv/bp are folded on the host in kernel()."""
    cpat = np.empty((P, T), dtype=np.float32)
    spat = np.empty((P, T), dtype=np.float32)
    inv_freq = (10000.0 ** (-(np.arange(32, dtype=np.float64)) / 32.0))
    ang = np.arange(T, dtype=np.float64)[None, :] * inv_freq[:, None]  # [32,T]
    cos32 = (np.cos(ang) / WS).astype(np.float32)
    sin32 = (np.sin(ang) / WS).astype(np.float32)
    for blk in range(4):
        cpat[blk * 32:(blk + 1) * 32] = cos32
        spat[blk * 32:(blk + 1) * 32] = sin32 if blk % 2 == 0 else -sin32

    jj = np.arange(P)[:, None]
    ii = np.arange(P)[None, :]
    mk = (jj <= ii).astype(np.float32)

    in_maps = []
    for c in range(n_cores):
        b, g = divmod(c, n_cores // 2)
        heads = [HPC * g + i for i in range(HPC)]
        qk_rows = []
        v_rows = []
        for h in heads:
            base = D * h
            qk_rows += [base + 2 * i for i in range(32)]
            qk_rows += [base + 2 * i + 1 for i in range(32)]
            v_rows += list(range(base, base + D))
        import ml_dtypes
        wqk = (np.concatenate(
            [Wq[qk_rows, :].T, Wk[qk_rows, :].T],
            axis=1) * WS).astype(ml_dtypes.bfloat16)
        wv = np.ascontiguousarray(Wv[v_rows, :].T).astype(ml_dtypes.bfloat16)
        # wp planes: [128, 2*768]; plane i col c row p = Wp[c, vch(128i+p)]
        wp2 = np.zeros((P, 2 * C), dtype=np.float32)
        wp_s = Wp[:, v_rows].T.astype(np.float32)   # [192, 768]
        wp2[:, 0:C] = wp_s[0:P, :]
        wp2[0:DQ - P, C:2 * C] = wp_s[P:DQ, :]
        import ml_dtypes
        xT = np.ascontiguousarray(x[b].T).astype(ml_dtypes.bfloat16)
        im = {
            "xT": xT, "wqk": np.ascontiguousarray(wqk), "wv": wv, "wp": wp2,
            "cpat": cpat.astype(ml_dtypes.bfloat16),
            "spat": spat.astype(ml_dtypes.bfloat16),
            "mk": mk.astype(ml_dtypes.bfloat16),
        }
        in_maps.append(im)
    return in_maps


_nc_cache = {}


def run(x, Wq, bq, Wk, bk, Wv, bv, Wp, bp, T=4096, n_cores=8, trace=False,
        trace_cores=None):
    assert not (np.any(bq) or np.any(bk)), "nonzero q/k bias unsupported"
    key = (T, n_cores)
    if key not in _nc_cache:
        _nc_cache[key] = build(T=T, n_cores=n_cores)
    nc = _nc_cache[key]
    in_maps = make_core_inputs(x, Wq, bq, Wk, bk, Wv, bv, Wp, bp,
                               T=T, n_cores=n_cores)
    res = run_bass_kernel_spmd(nc, in_maps, list(range(n_cores)), trace=trace,
                               trace_cores=trace_cores)
    B = 2
    out = np.zeros((B, T, C), dtype=np.float32)
    for c in range(n_cores):
        b = c // (n_cores // 2)
        out[b] += res.results[c]["outT"].T
    # host-folded bias terms: softmax rows sum to 1, so the v bias passes
    # through attention unchanged: y = att@v + bv  =>  out += bv @ Wp.T + bp
    out += (bv.astype(np.float32) @ Wp.T.astype(np.float32) + bp)[None, None, :]
    return out, res


def kernel(**inputs):
    inputs = {k: np.asarray(v) for k, v in inputs.items()}
    out, _ = run(**inputs)
    return out
